# revision 36
# baseline (speedup 1.0000x reference)
"""Multi-head attention on 8 trn2 NeuronCores, head-parallel (2 heads/core).

Math per head h (reference semantics):
  Q = query @ Wq[h] + bq[h];  K = key @ Wk[h] + bk[h];  V = query @ Wv[h]
  P = exp(Q K^T / sqrt(D));  alpha = P / rowsum(P)
  ctx = alpha @ V;  y_h = (ctx @ Wp[h] + bp[h]) @ Wo[h]
  out = sum_h y_h + bo      (bv/bp/bo terms folded into a host-side bias)

Device-side v2 (fp8 PV + interleaved projection):
  Projections in bf16 (QT/KT [d, tok]; V in natural [tok, d] layout quantized
  to fp8-e4m3).  Attention: scores via bf16 matmuls, exp on ACT writing
  fp8-e4m3 pexp, PV and the rowsum (ones-stationary) via fp8 DoubleRow
  matmuls at 2x rate.  Normalization via reciprocal_approx_fast + mul.
  y^T partial = sum_{h} wh_h^T @ ctxn_h + bias, wh = Wp@Wo host-folded.
  Projection matmuls of batch b+1 are interleaved into attention of batch b
  so the PE never idles on ACT.  ReduceScatter over 8 cores per (b, qbp).
"""

import sys

if "/opt/trn_rl_repo" not in sys.path:
    sys.path.insert(0, "/opt/trn_rl_repo")

import ml_dtypes
import numpy as np

import concourse.mybir as mybir
import concourse.tile as tile
from concourse import bacc
from concourse.bass_utils import run_bass_kernel_spmd

B, S = 4, 2048
IN, D, H = 1024, 128, 16
NCORES = 8
HPC = H // NCORES  # heads per core
NCH = IN // 128  # input chunks
TB = 512  # projection token block
NTB = S // TB
QB = 512  # attention query block
NKT = S // 128  # attention key tiles (128 tokens)
NKT2 = NKT // 2  # key tile pairs (256 tokens, DoubleRow)
ESH = D // NCORES  # output shard rows per core

f32 = mybir.dt.float32
f32r = mybir.dt.float32r
bf16 = mybir.dt.bfloat16
f8 = mybir.dt.float8e4
AF = mybir.ActivationFunctionType
DR = mybir.MatmulPerfMode.DoubleRow

_cache = {}


def build():
    nc = bacc.Bacc(None, target_bir_lowering=False, num_devices=NCORES)

    qT = nc.dram_tensor("qT", [B, IN, S], f8, kind="ExternalInput")
    kT = nc.dram_tensor("kT", [B, IN, S], f8, kind="ExternalInput")
    wq = nc.dram_tensor("wq", [HPC, IN, D], f8, kind="ExternalInput")
    wk = nc.dram_tensor("wk", [HPC, IN, D], f8, kind="ExternalInput")
    wv = nc.dram_tensor("wv", [128, NCH, HPC, D], f8, kind="ExternalInput")
    wh = nc.dram_tensor("wh", [HPC, D, D], f32r, kind="ExternalInput")
    bqT = nc.dram_tensor("bqT", [D, HPC], f32, kind="ExternalInput")
    bkT = nc.dram_tensor("bkT", [D, HPC], f32, kind="ExternalInput")
    biasE = nc.dram_tensor("biasE", [D, 1], f32, kind="ExternalInput")
    ones8 = nc.dram_tensor("ones8", [128, 2, 128], f8, kind="ExternalInput")

    out_y = nc.dram_tensor("out_y", [ESH, B * S], bf16, kind="ExternalOutput")
    y_bounce = [
        [nc.dram_tensor(f"y_bounce{b}_{q}", [D, S // 2], bf16) for q in range(2)]
        for b in range(B)
    ]
    y_shard = [
        [nc.dram_tensor(f"y_shard{b}_{q}", [ESH, S // 2], bf16) for q in range(2)]
        for b in range(B)
    ]

    # fp8 weight scaling: weights are stored *WS, so QT/KT carry a factor of
    # WS each -> fold 1/WS^2 into the exp scale; wh carries the 1/WS for V.
    WS = 32.0
    scale = 1.0 / (float(np.sqrt(D)) * WS * WS)

    with tile.TileContext(nc) as tc:
        with (
            tc.tile_pool(name="const", bufs=1) as cpool,
            tc.tile_pool(name="xch", bufs=4) as xch,
            tc.tile_pool(name="qkv", bufs=2) as qkv,
            tc.tile_pool(name="pexpp", bufs=2) as pexpp,
            tc.tile_pool(name="work", bufs=2) as work,
            tc.tile_pool(name="psq", bufs=2, space="PSUM") as psq,
            tc.tile_pool(name="psc", bufs=2, space="PSUM") as psc,
            tc.tile_pool(name="psp", bufs=2, space="PSUM") as psp,
        ):
            # ---- resident constants (tiles first; DMAs ordered for startup) ----
            wq_sb = cpool.tile([128, HPC, NCH, D], f8, tag="wq_sb")
            wk_sb = cpool.tile([128, HPC, NCH, D], f8, tag="wk_sb")
            # V weights pre-arranged host-side as [p, c, h, d] so a c-pair
            # slice is the DoubleRow moving operand [2, HPC*D]
            wv_sb = cpool.tile([128, NCH, HPC, D], f8, tag="wv_sb")
            wh_sb = cpool.tile([128, HPC, D], f32r, tag="wh_sb")
            bq_sb = cpool.tile([128, HPC], f32, tag="bq_sb")
            bk_sb = cpool.tile([128, HPC], f32, tag="bk_sb")
            biasE_sb = cpool.tile([128, 1], f32, tag="biasE_sb")
            ones8_sb = cpool.tile([128, 2, 128], f8, tag="ones8_sb")

            def dma_consts_early():
                # spread across engine DMA queues so the startup loads overlap
                nc.scalar.dma_start(
                    wq_sb[:], wq[:].rearrange("h (c p) d -> p h c d", p=128)
                )
                nc.gpsimd.dma_start(
                    wk_sb[:], wk[:].rearrange("h (c p) d -> p h c d", p=128)
                )
                nc.gpsimd.dma_start(bq_sb[:], bqT[:])
                nc.gpsimd.dma_start(bk_sb[:], bkT[:])

            def dma_consts_late():
                nc.scalar.dma_start(wv_sb[:], wv[:])
                nc.gpsimd.dma_start(wh_sb[:], wh[:].rearrange("h d e -> d h e"))
                nc.gpsimd.dma_start(biasE_sb[:], biasE[:])
                nc.gpsimd.dma_start(ones8_sb[:], ones8[:])
                # prewarm the ACT exp table while projections run
                warm = cpool.tile([128, 1], f32, tag="warm")
                nc.scalar.activation(warm[:], biasE_sb[:], AF.Exp)

            QTd, KTd, V8d = {}, {}, {}

            def make_proj(b):
                """Return (dma_fns per tb, unit fns) for projecting batch b."""
                QT = QTd[b] = [
                    qkv.tile([128, S], bf16, tag=f"QT{h}", name=f"QT{h}")
                    for h in range(HPC)
                ]
                KTs = KTd[b] = [
                    qkv.tile([128, S], bf16, tag=f"KT{h}", name=f"KT{h}")
                    for h in range(HPC)
                ]
                V8 = V8d[b] = [
                    qkv.tile([128, NKT2, 2, 128], f8, tag=f"V8{h}", name=f"V8{h}")
                    for h in range(HPC)
                ]
                chq_t, chk_t = {}, {}

                def dma_q(tb):
                    chq = xch.tile([128, NCH, TB], f8, tag="chq", name="chq")
                    nc.sync.dma_start(
                        chq[:],
                        qT[b, :, tb * TB : (tb + 1) * TB].rearrange(
                            "(c p) n -> p c n", p=128
                        ),
                    )
                    chq_t[tb] = chq

                def dma_k(tb):
                    chk = xch.tile([128, NCH, TB], f8, tag="chk", name="chk")
                    nc.sync.dma_start(
                        chk[:],
                        kT[b, :, tb * TB : (tb + 1) * TB].rearrange(
                            "(c p) n -> p c n", p=128
                        ),
                    )
                    chk_t[tb] = chk

                def dma_tb(tb):
                    dma_q(tb)
                    dma_k(tb)

                def unit_qk(tb, h, w_sb, b_sb, dst, chd):
                    st = {}

                    def first():
                        chs = chd[tb]
                        pp = st["pp"] = psp.tile([128, TB], f32, tag="pP", name="pp")
                        for cp in range(2):
                            nc.tensor.matmul(
                                pp[:],
                                w_sb[:, h, 2 * cp : 2 * cp + 2, :],
                                chs[:, 2 * cp : 2 * cp + 2, :],
                                start=(cp == 0),
                                stop=False,
                                perf_mode=DR,
                            )

                    def second():
                        chs = chd[tb]
                        pp = st["pp"]
                        for cp in range(2, 4):
                            nc.tensor.matmul(
                                pp[:],
                                w_sb[:, h, 2 * cp : 2 * cp + 2, :],
                                chs[:, 2 * cp : 2 * cp + 2, :],
                                start=False,
                                stop=(cp == 3),
                                perf_mode=DR,
                            )
                        with nc.allow_low_precision(reason="bf16 QT/KT"):
                            nc.vector.tensor_scalar_add(
                                dst[h][:, tb * TB : (tb + 1) * TB],
                                pp[:],
                                b_sb[:, h : h + 1],
                            )

                    return [first, second]

                def unit_v(tb, tpair):
                    st = {}

                    def half(tt):
                        t = tpair * 2 + tt
                        chs = chq_t[tb]
                        if tt == 0:
                            pvt = st["pvt"] = psp.tile(
                                [128, TB], f32, tag="pP", name="pvt"
                            )
                        else:
                            pvt = st["pvt"]
                        for cp in range(NCH // 2):
                            nc.tensor.matmul(
                                pvt[:, tt * 256 : (tt + 1) * 256],
                                chs[:, 2 * cp : 2 * cp + 2, t * 128 : (t + 1) * 128],
                                wv_sb[:, 2 * cp : 2 * cp + 2, :, :],
                                start=(cp == 0),
                                stop=(cp == NCH // 2 - 1),
                                perf_mode=DR,
                            )
                        kt = tb * 4 + t
                        for h in range(HPC):
                            with nc.allow_low_precision(reason="fp8 V"):
                                nc.vector.tensor_copy(
                                    V8[h][:, kt // 2, kt % 2, :],
                                    pvt[:, tt * 256 + h * D : tt * 256 + (h + 1) * D],
                                )

                    return [lambda: half(0), lambda: half(1)]

                dmas = [lambda tb=tb: dma_tb(tb) for tb in range(NTB)]
                units = []
                for tb in range(NTB):
                    units += unit_qk(tb, 0, wq_sb, bq_sb, QT, chq_t)
                    units += unit_qk(tb, 1, wq_sb, bq_sb, QT, chq_t)
                    units += unit_qk(tb, 0, wk_sb, bk_sb, KTs, chk_t)
                    units += unit_qk(tb, 1, wk_sb, bk_sb, KTs, chk_t)
                    units += unit_v(tb, 0)
                    units += unit_v(tb, 1)
                kv_first = []
                q_units = []
                for tb in range(NTB):
                    kv_first += unit_qk(tb, 0, wk_sb, bk_sb, KTs, chk_t)
                    kv_first += unit_qk(tb, 1, wk_sb, bk_sb, KTs, chk_t)
                    kv_first += unit_v(tb, 0)
                    kv_first += unit_v(tb, 1)
                    q_units.append(
                        unit_qk(tb, 0, wq_sb, bq_sb, QT, chq_t)
                        + unit_qk(tb, 1, wq_sb, bq_sb, QT, chq_t)
                    )
                return dmas, units, kv_first, q_units, dma_q, dma_k

            def qk_sub(pexp, bb, h, qbp, kp, j):
                """Scores + exp for one 128-token key subtile."""
                QT, KTs = QTd[bb][h], KTd[bb][h]
                q0 = qbp * 2 * QB
                kt = kp * 2 + j
                ksl = slice(kt * 128, (kt + 1) * 128)
                ps2 = psq.tile([128, 2 * QB], f32, tag="pS", name="ps2")
                nc.tensor.matmul(
                    ps2[:, :QB], KTs[:, ksl], QT[:, q0 : q0 + QB],
                    start=True, stop=True,
                )
                nc.tensor.matmul(
                    ps2[:, QB:], KTs[:, ksl], QT[:, q0 + QB : q0 + 2 * QB],
                    start=True, stop=True,
                )
                nc.scalar.activation(pexp[:, kp, j, :], ps2[:], AF.Exp, scale=scale)

            def prs_pair():
                prs0 = psp.tile([128, QB], f32, tag="pP", name="prs0")
                prs1 = psp.tile([128, QB], f32, tag="pP", name="prs1")
                return prs0, prs1

            def prs_mms(prs0, prs1, pexp, kp):
                nc.tensor.matmul(
                    prs0[:], ones8_sb[:], pexp[:, kp, :, :QB],
                    start=(kp == 0), stop=(kp == NKT2 - 1), perf_mode=DR,
                )
                nc.tensor.matmul(
                    prs1[:], ones8_sb[:], pexp[:, kp, :, QB:],
                    start=(kp == 0), stop=(kp == NKT2 - 1), perf_mode=DR,
                )

            def section(b, h, qbp, pull, pre, nxt, last_batch):
                """Attention for head h, query pair-block qbp (1024 qs).

                `pre`: pexp tile whose (kp=0, j=0) subtile is already emitted.
                `nxt`: (b', h', qbp') of the following section - its first
                subtile is emitted here (before the rowsum burst) so ACT
                stays fed; returns (ctx, carry_pexp).
                """
                V8 = V8d[b][h]
                if pre is not None:
                    pexp = pre
                else:
                    pexp = pexpp.tile(
                        [128, NKT2, 2, 2 * QB], f8, tag="pexp", name="pexp"
                    )
                    qk_sub(pexp, b, h, qbp, 0, 0)
                pctx0 = psc.tile([128, QB], f32, tag="pC", name="pctx0")
                pctx1 = psc.tile([128, QB], f32, tag="pC", name="pctx1")
                if last_batch:
                    prs0, prs1 = prs_pair()
                for kp in range(NKT2):
                    pull()
                    for j in range(2):
                        if kp == 0 and j == 0:
                            continue
                        qk_sub(pexp, b, h, qbp, kp, j)
                        if j == 0:
                            pull()
                    nc.tensor.matmul(
                        pctx0[:],
                        V8[:, kp],
                        pexp[:, kp, :, :QB],
                        start=(kp == 0),
                        stop=(kp == NKT2 - 1),
                        perf_mode=DR,
                    )
                    nc.tensor.matmul(
                        pctx1[:],
                        V8[:, kp],
                        pexp[:, kp, :, QB:],
                        start=(kp == 0),
                        stop=(kp == NKT2 - 1),
                        perf_mode=DR,
                    )
                    if last_batch:
                        prs_mms(prs0, prs1, pexp, kp)
                # warm the next section's first subtile before the rowsum burst
                carry = None
                if nxt is not None:
                    carry = pexpp.tile(
                        [128, NKT2, 2, 2 * QB], f8, tag="pexp", name="pexp"
                    )
                    qk_sub(carry, *nxt, 0, 0)
                if not last_batch:
                    prs0, prs1 = prs_pair()
                    for kp in range(NKT2):
                        prs_mms(prs0, prs1, pexp, kp)
                rsbr = work.tile([128, 2 * QB], f32, tag="rsbr", name="rsbr")
                ctx = work.tile([128, 2 * QB], f32r, tag=f"ctx{h}", name="ctx")
                nc.vector.reciprocal_approx_fast(out=rsbr[:, :QB], in_=prs0[:])
                with nc.allow_low_precision(reason="f32r PE operand"):
                    nc.vector.tensor_mul(ctx[:, :QB], pctx0[:], rsbr[:, :QB])
                nc.vector.reciprocal_approx_fast(out=rsbr[:, QB:], in_=prs1[:])
                with nc.allow_low_precision(reason="f32r PE operand"):
                    nc.vector.tensor_mul(ctx[:, QB:], pctx1[:], rsbr[:, QB:])
                return ctx, carry

            def attn_batch(b, dmas, units, pre):
                uit = iter(units)
                nunits = len(units)
                emitted = [0]
                slot = [0]

                def pull():
                    want = (slot[0] + 1) * nunits // (8 * NKT2)
                    while emitted[0] < want:
                        next(uit)()
                        emitted[0] += 1
                    slot[0] += 1

                last_batch = b == B - 1
                if dmas:
                    dmas[0]()
                    dmas[1]()
                for qbp in range(2):
                    ctxs = []
                    for h in range(HPC):
                        s_idx = qbp * 2 + h
                        if s_idx + 2 < len(dmas):
                            dmas[s_idx + 2]()
                        if s_idx == 3:
                            # the cross-batch warm reads QT/KT of batch b+1:
                            # force the remaining projection units out first
                            # (they precede the warm in the PE FIFO).
                            for fn in uit:
                                fn()
                                emitted[0] += 1
                        if s_idx < 3:
                            nxt = (b, (h + 1) % HPC, qbp + (h == HPC - 1))
                        elif not last_batch:
                            nxt = (b + 1, 0, 0)
                        else:
                            nxt = None
                        ctx, pre = section(b, h, qbp, pull, pre, nxt, last_batch)
                        ctxs.append(ctx)
                    # output projection for this query pair-block
                    ytile = work.tile([128, 2 * QB], bf16, tag="ytile", name="ytile")
                    for half, hsl in ((0, slice(0, QB)), (1, slice(QB, 2 * QB))):
                        pz = psc.tile([128, QB], f32, tag="pC", name="pz")
                        for h in range(HPC):
                            nc.tensor.matmul(
                                pz[:],
                                wh_sb[:, h, :],
                                ctxs[h][:, hsl],
                                start=(h == 0),
                                stop=(h == HPC - 1),
                            )
                        with nc.allow_low_precision(reason="bf16 RS payload"):
                            nc.vector.tensor_scalar_add(
                                ytile[:, hsl], pz[:], biasE_sb[:, 0:1]
                            )
                        nc.gpsimd.dma_start(y_bounce[b][qbp][:, hsl], ytile[:, hsl])
                    nc.gpsimd.collective_compute(
                        "ReduceScatter",
                        mybir.AluOpType.add,
                        replica_groups=[list(range(NCORES))],
                        ins=[y_bounce[b][qbp][:].opt()],
                        outs=[y_shard[b][qbp][:].opt()],
                    )
                    nc.sync.dma_start(
                        out_y[:, b * S + qbp * (S // 2) : b * S + (qbp + 1) * (S // 2)],
                        y_shard[b][qbp][:],
                    )
                return pre

            # ---- prologue: project batch 0 (K and V first; attention can
            # start once Q's first two token blocks are done) ----
            dma_consts_early()
            _, _, kv0, q0, dq0, dk0 = make_proj(0)
            dk0(0)
            dk0(1)
            dma_consts_late()
            dq0(0)
            dk0(2)
            dq0(1)
            dk0(3)
            for tb in range(NTB):
                if tb + 2 < NTB:
                    dq0(tb + 2)
                for u in kv0[tb * 8 : (tb + 1) * 8]:
                    u()
            for u in q0[0] + q0[1]:
                u()
            leftover = q0[2] + q0[3]

            # warm attn(0)'s very first scores subtile at the prologue end
            pre = pexpp.tile([128, NKT2, 2, 2 * QB], f8, tag="pexp", name="pexp")
            qk_sub(pre, 0, 0, 0, 0, 0)

            for b in range(B):
                if b + 1 < B:
                    dmas, units, _, _, _, _ = make_proj(b + 1)
                else:
                    dmas, units = [], []
                pre = attn_batch(b, dmas, leftover + units, pre)
                leftover = []

    nc.compile()
    return nc


def kernel(**inputs):
    query = np.asarray(inputs["query"], np.float32)
    key = np.asarray(inputs["key"], np.float32)
    Wq, bq = np.asarray(inputs["Wq"], np.float32), np.asarray(inputs["bq"], np.float32)
    Wk, bk = np.asarray(inputs["Wk"], np.float32), np.asarray(inputs["bk"], np.float32)
    Wv = np.asarray(inputs["Wv"], np.float32)
    bv = np.asarray(inputs["bv"], np.float32)
    Wp, bp = np.asarray(inputs["Wp"], np.float32), np.asarray(inputs["bp"], np.float32)
    Wo, bo = np.asarray(inputs["Wo"], np.float32), np.asarray(inputs["bo"], np.float32)

    e4 = ml_dtypes.float8_e4m3fn if hasattr(ml_dtypes, "float8_e4m3fn") else ml_dtypes.float8_e4m3
    WS = 32.0

    qT_f8 = np.ascontiguousarray(query.transpose(0, 2, 1)).astype(e4)
    kT_f8 = np.ascontiguousarray(key.transpose(0, 2, 1)).astype(e4)

    if "nc" not in _cache:
        _cache["nc"] = build()
    nc = _cache["nc"]

    in_maps = []
    for i in range(NCORES):
        hs = slice(i * HPC, (i + 1) * HPC)
        Wo_h = Wo.reshape(H, D, D)  # rows of Wo per head
        wh = np.einsum(
            "hde,hef->hdf",
            Wp[hs].astype(np.float64),
            Wo_h[hs].astype(np.float64),
        ).astype(np.float32)
        bias = (
            np.einsum("hd,hdf->f", bv[hs].astype(np.float64), wh.astype(np.float64))
            + np.einsum(
                "hd,hdf->f", bp[hs].astype(np.float64), Wo_h[hs].astype(np.float64)
            )
            + bo.astype(np.float64) / NCORES
        ).astype(np.float32)
        in_maps.append(
            {
                "qT": qT_f8,
                "kT": kT_f8,
                "wq": np.ascontiguousarray(Wq[hs] * WS).astype(e4),
                "wk": np.ascontiguousarray(Wk[hs] * WS).astype(e4),
                "wv": np.ascontiguousarray(
                    (Wv[hs] * WS).reshape(HPC, NCH, 128, D).transpose(2, 1, 0, 3)
                ).astype(e4),
                "wh": wh / WS,
                "bqT": np.ascontiguousarray(bq[hs].T) * WS,
                "bkT": np.ascontiguousarray(bk[hs].T) * WS,
                "biasE": bias.reshape(D, 1),
                "ones8": np.ones((128, 2, 128), e4),
            }
        )

    res = run_bass_kernel_spmd(nc, in_maps, core_ids=list(range(NCORES)))
    _cache["last_result"] = res
    yT = np.concatenate(
        [np.asarray(res.results[i]["out_y"]).astype(np.float32) for i in range(NCORES)],
        axis=0,
    )
    return np.ascontiguousarray(yT.T).reshape(B, S, D)


# revision 39
# speedup vs baseline: 1.0081x; 1.0081x over previous
"""Multi-head attention on 8 trn2 NeuronCores, head-parallel (2 heads/core).

Math per head h (reference semantics):
  Q = query @ Wq[h] + bq[h];  K = key @ Wk[h] + bk[h];  V = query @ Wv[h]
  P = exp(Q K^T / sqrt(D));  alpha = P / rowsum(P)
  ctx = alpha @ V;  y_h = (ctx @ Wp[h] + bp[h]) @ Wo[h]
  out = sum_h y_h + bo      (bv/bp/bo terms folded into a host-side bias)

Device-side v2 (fp8 PV + interleaved projection):
  Projections in bf16 (QT/KT [d, tok]; V in natural [tok, d] layout quantized
  to fp8-e4m3).  Attention: scores via bf16 matmuls, exp on ACT writing
  fp8-e4m3 pexp, PV and the rowsum (ones-stationary) via fp8 DoubleRow
  matmuls at 2x rate.  Normalization via reciprocal_approx_fast + mul.
  y^T partial = sum_{h} wh_h^T @ ctxn_h + bias, wh = Wp@Wo host-folded.
  Projection matmuls of batch b+1 are interleaved into attention of batch b
  so the PE never idles on ACT.  ReduceScatter over 8 cores per (b, qbp).
"""

import sys

if "/opt/trn_rl_repo" not in sys.path:
    sys.path.insert(0, "/opt/trn_rl_repo")

import ml_dtypes
import numpy as np

import concourse.mybir as mybir
import concourse.tile as tile
from concourse import bacc
from concourse.bass_utils import run_bass_kernel_spmd

B, S = 4, 2048
IN, D, H = 1024, 128, 16
NCORES = 8
HPC = H // NCORES  # heads per core
NCH = IN // 128  # input chunks
TB = 512  # projection token block
NTB = S // TB
QB = 512  # attention query block
NKT = S // 128  # attention key tiles (128 tokens)
NKT2 = NKT // 2  # key tile pairs (256 tokens, DoubleRow)
ESH = D // NCORES  # output shard rows per core

f32 = mybir.dt.float32
f32r = mybir.dt.float32r
bf16 = mybir.dt.bfloat16
f8 = mybir.dt.float8e4
AF = mybir.ActivationFunctionType
DR = mybir.MatmulPerfMode.DoubleRow

_cache = {}


def build():
    nc = bacc.Bacc(None, target_bir_lowering=False, num_devices=NCORES)

    qT = nc.dram_tensor("qT", [B, IN, S], f8, kind="ExternalInput")
    kT = nc.dram_tensor("kT", [B, IN, S], f8, kind="ExternalInput")
    wq = nc.dram_tensor("wq", [HPC, IN, D], f8, kind="ExternalInput")
    wk = nc.dram_tensor("wk", [HPC, IN, D], f8, kind="ExternalInput")
    wv = nc.dram_tensor("wv", [128, NCH, HPC, D], f8, kind="ExternalInput")
    wh = nc.dram_tensor("wh", [HPC, D, D], f32r, kind="ExternalInput")
    bqT = nc.dram_tensor("bqT", [D, HPC], f32, kind="ExternalInput")
    bkT = nc.dram_tensor("bkT", [D, HPC], f32, kind="ExternalInput")
    biasE = nc.dram_tensor("biasE", [D, 1], f32, kind="ExternalInput")
    ones8 = nc.dram_tensor("ones8", [128, 2, 128], f8, kind="ExternalInput")

    out_y = nc.dram_tensor("out_y", [ESH, B * S], bf16, kind="ExternalOutput")
    y_bounce = [
        [nc.dram_tensor(f"y_bounce{b}_{q}", [D, S // 2], bf16) for q in range(2)]
        for b in range(B)
    ]
    y_shard = [
        [nc.dram_tensor(f"y_shard{b}_{q}", [ESH, S // 2], bf16) for q in range(2)]
        for b in range(B)
    ]

    # fp8 weight scaling: weights are stored *WS, so QT/KT carry a factor of
    # WS each -> fold 1/WS^2 into the exp scale; wh carries the 1/WS for V.
    WS = 32.0
    scale = 1.0 / (float(np.sqrt(D)) * WS * WS)

    with tile.TileContext(nc) as tc:
        with (
            tc.tile_pool(name="const", bufs=1) as cpool,
            tc.tile_pool(name="xch", bufs=4) as xch,
            tc.tile_pool(name="qkv", bufs=2) as qkv,
            tc.tile_pool(name="pexpp", bufs=2) as pexpp,
            tc.tile_pool(name="work", bufs=2) as work,
            tc.tile_pool(name="psq", bufs=2, space="PSUM") as psq,
            tc.tile_pool(name="psc", bufs=2, space="PSUM") as psc,
            tc.tile_pool(name="psp", bufs=2, space="PSUM") as psp,
        ):
            # ---- resident constants (tiles first; DMAs ordered for startup) ----
            wq_sb = cpool.tile([128, HPC, NCH, D], f8, tag="wq_sb")
            wk_sb = cpool.tile([128, HPC, NCH, D], f8, tag="wk_sb")
            # V weights pre-arranged host-side as [p, c, h, d] so a c-pair
            # slice is the DoubleRow moving operand [2, HPC*D]
            wv_sb = cpool.tile([128, NCH, HPC, D], f8, tag="wv_sb")
            wh_sb = cpool.tile([128, HPC, D], f32r, tag="wh_sb")
            bq_sb = cpool.tile([128, HPC], f32, tag="bq_sb")
            bk_sb = cpool.tile([128, HPC], f32, tag="bk_sb")
            biasE_sb = cpool.tile([128, 1], f32, tag="biasE_sb")
            ones8_sb = cpool.tile([128, 2, 128], f8, tag="ones8_sb")

            def dma_consts_early():
                # spread across engine DMA queues so the startup loads overlap
                nc.scalar.dma_start(
                    wq_sb[:], wq[:].rearrange("h (c p) d -> p h c d", p=128)
                )
                nc.gpsimd.dma_start(
                    wk_sb[:], wk[:].rearrange("h (c p) d -> p h c d", p=128)
                )
                nc.gpsimd.dma_start(bq_sb[:], bqT[:])
                nc.gpsimd.dma_start(bk_sb[:], bkT[:])

            def dma_consts_late():
                nc.scalar.dma_start(wv_sb[:], wv[:])
                nc.gpsimd.dma_start(wh_sb[:], wh[:].rearrange("h d e -> d h e"))
                nc.gpsimd.dma_start(biasE_sb[:], biasE[:])
                nc.gpsimd.dma_start(ones8_sb[:], ones8[:])
                # prewarm the ACT exp table while projections run
                warm = cpool.tile([128, 1], f32, tag="warm")
                nc.scalar.activation(warm[:], biasE_sb[:], AF.Exp)

            QTd, KTd, V8d = {}, {}, {}

            def make_proj(b):
                """Return (dma_fns per tb, unit fns) for projecting batch b."""
                QT = QTd[b] = [
                    qkv.tile([128, S], bf16, tag=f"QT{h}", name=f"QT{h}")
                    for h in range(HPC)
                ]
                KTs = KTd[b] = [
                    qkv.tile([128, S], bf16, tag=f"KT{h}", name=f"KT{h}")
                    for h in range(HPC)
                ]
                V8 = V8d[b] = [
                    qkv.tile([128, NKT2, 2, 128], f8, tag=f"V8{h}", name=f"V8{h}")
                    for h in range(HPC)
                ]
                chq_t, chk_t = {}, {}

                def dma_q(tb):
                    chq = xch.tile([128, NCH, TB], f8, tag="chq", name="chq")
                    nc.sync.dma_start(
                        chq[:],
                        qT[b, :, tb * TB : (tb + 1) * TB].rearrange(
                            "(c p) n -> p c n", p=128
                        ),
                    )
                    chq_t[tb] = chq

                def dma_k(tb):
                    chk = xch.tile([128, NCH, TB], f8, tag="chk", name="chk")
                    nc.sync.dma_start(
                        chk[:],
                        kT[b, :, tb * TB : (tb + 1) * TB].rearrange(
                            "(c p) n -> p c n", p=128
                        ),
                    )
                    chk_t[tb] = chk

                def dma_tb(tb):
                    dma_q(tb)
                    dma_k(tb)

                def unit_qk(tb, h, w_sb, b_sb, dst, chd):
                    st = {}

                    def first():
                        chs = chd[tb]
                        pp = st["pp"] = psp.tile([128, TB], f32, tag="pP", name="pp")
                        for cp in range(2):
                            nc.tensor.matmul(
                                pp[:],
                                w_sb[:, h, 2 * cp : 2 * cp + 2, :],
                                chs[:, 2 * cp : 2 * cp + 2, :],
                                start=(cp == 0),
                                stop=False,
                                perf_mode=DR,
                            )

                    def second():
                        chs = chd[tb]
                        pp = st["pp"]
                        for cp in range(2, 4):
                            nc.tensor.matmul(
                                pp[:],
                                w_sb[:, h, 2 * cp : 2 * cp + 2, :],
                                chs[:, 2 * cp : 2 * cp + 2, :],
                                start=False,
                                stop=(cp == 3),
                                perf_mode=DR,
                            )
                        with nc.allow_low_precision(reason="bf16 QT/KT"):
                            nc.vector.tensor_scalar_add(
                                dst[h][:, tb * TB : (tb + 1) * TB],
                                pp[:],
                                b_sb[:, h : h + 1],
                            )

                    return [first, second]

                def unit_v(tb, tpair):
                    st = {}

                    def half(tt):
                        t = tpair * 2 + tt
                        chs = chq_t[tb]
                        if tt == 0:
                            pvt = st["pvt"] = psp.tile(
                                [128, TB], f32, tag="pP", name="pvt"
                            )
                        else:
                            pvt = st["pvt"]
                        for cp in range(NCH // 2):
                            nc.tensor.matmul(
                                pvt[:, tt * 256 : (tt + 1) * 256],
                                chs[:, 2 * cp : 2 * cp + 2, t * 128 : (t + 1) * 128],
                                wv_sb[:, 2 * cp : 2 * cp + 2, :, :],
                                start=(cp == 0),
                                stop=(cp == NCH // 2 - 1),
                                perf_mode=DR,
                            )
                        kt = tb * 4 + t
                        for h in range(HPC):
                            with nc.allow_low_precision(reason="fp8 V"):
                                nc.vector.tensor_copy(
                                    V8[h][:, kt // 2, kt % 2, :],
                                    pvt[:, tt * 256 + h * D : tt * 256 + (h + 1) * D],
                                )

                    return [lambda: half(0), lambda: half(1)]

                dmas = [lambda tb=tb: dma_tb(tb) for tb in range(NTB)]
                units = []
                for tb in range(NTB):
                    units += unit_qk(tb, 0, wq_sb, bq_sb, QT, chq_t)
                    units += unit_qk(tb, 1, wq_sb, bq_sb, QT, chq_t)
                    units += unit_qk(tb, 0, wk_sb, bk_sb, KTs, chk_t)
                    units += unit_qk(tb, 1, wk_sb, bk_sb, KTs, chk_t)
                    units += unit_v(tb, 0)
                    units += unit_v(tb, 1)
                uq = {
                    (tb, h): unit_qk(tb, h, wq_sb, bq_sb, QT, chq_t)
                    for tb in range(NTB)
                    for h in range(HPC)
                }
                uk = {
                    (tb, h): unit_qk(tb, h, wk_sb, bk_sb, KTs, chk_t)
                    for tb in range(NTB)
                    for h in range(HPC)
                }
                uv = {
                    (tb, tp): unit_v(tb, tp) for tb in range(NTB) for tp in range(2)
                }
                return dmas, units, uq, uk, uv, dma_q, dma_k

            def qk_sub(pexp, bb, h, qbp, kp, j):
                """Scores + exp for one 128-token key subtile."""
                QT, KTs = QTd[bb][h], KTd[bb][h]
                q0 = qbp * 2 * QB
                kt = kp * 2 + j
                ksl = slice(kt * 128, (kt + 1) * 128)
                ps2 = psq.tile([128, 2 * QB], f32, tag="pS", name="ps2")
                nc.tensor.matmul(
                    ps2[:, :QB], KTs[:, ksl], QT[:, q0 : q0 + QB],
                    start=True, stop=True,
                )
                nc.tensor.matmul(
                    ps2[:, QB:], KTs[:, ksl], QT[:, q0 + QB : q0 + 2 * QB],
                    start=True, stop=True,
                )
                nc.scalar.activation(pexp[:, kp, j, :], ps2[:], AF.Exp, scale=scale)

            def prs_pair():
                prs0 = psp.tile([128, QB], f32, tag="pP", name="prs0")
                prs1 = psp.tile([128, QB], f32, tag="pP", name="prs1")
                return prs0, prs1

            def prs_mms(prs0, prs1, pexp, kp):
                nc.tensor.matmul(
                    prs0[:], ones8_sb[:], pexp[:, kp, :, :QB],
                    start=(kp == 0), stop=(kp == NKT2 - 1), perf_mode=DR,
                )
                nc.tensor.matmul(
                    prs1[:], ones8_sb[:], pexp[:, kp, :, QB:],
                    start=(kp == 0), stop=(kp == NKT2 - 1), perf_mode=DR,
                )

            def section(b, h, qbp, pull, pre, nxt, last_batch):
                """Attention for head h, query pair-block qbp (1024 qs).

                `pre`: pexp tile whose (kp=0, j=0) subtile is already emitted.
                `nxt`: (b', h', qbp') of the following section - its first
                subtile is emitted here (before the rowsum burst) so ACT
                stays fed; returns (ctx, carry_pexp).
                """
                V8 = V8d[b][h]
                if pre is not None:
                    pexp = pre
                else:
                    pexp = pexpp.tile(
                        [128, NKT2, 2, 2 * QB], f8, tag="pexp", name="pexp"
                    )
                    qk_sub(pexp, b, h, qbp, 0, 0)
                pctx0 = psc.tile([128, QB], f32, tag="pC", name="pctx0")
                pctx1 = psc.tile([128, QB], f32, tag="pC", name="pctx1")
                if last_batch:
                    prs0, prs1 = prs_pair()
                for kp in range(NKT2):
                    pull()
                    for j in range(2):
                        if kp == 0 and j == 0:
                            continue
                        qk_sub(pexp, b, h, qbp, kp, j)
                        if j == 0:
                            pull()
                    nc.tensor.matmul(
                        pctx0[:],
                        V8[:, kp],
                        pexp[:, kp, :, :QB],
                        start=(kp == 0),
                        stop=(kp == NKT2 - 1),
                        perf_mode=DR,
                    )
                    nc.tensor.matmul(
                        pctx1[:],
                        V8[:, kp],
                        pexp[:, kp, :, QB:],
                        start=(kp == 0),
                        stop=(kp == NKT2 - 1),
                        perf_mode=DR,
                    )
                    if last_batch:
                        prs_mms(prs0, prs1, pexp, kp)
                # warm the next section's first subtile before the rowsum burst
                carry = None
                if nxt is not None:
                    carry = pexpp.tile(
                        [128, NKT2, 2, 2 * QB], f8, tag="pexp", name="pexp"
                    )
                    qk_sub(carry, *nxt, 0, 0)
                if not last_batch:
                    prs0, prs1 = prs_pair()
                    for kp in range(NKT2):
                        prs_mms(prs0, prs1, pexp, kp)
                rsbr = work.tile([128, 2 * QB], f32, tag="rsbr", name="rsbr")
                ctx = work.tile([128, 2 * QB], f32r, tag=f"ctx{h}", name="ctx")
                nc.vector.reciprocal_approx_fast(out=rsbr[:, :QB], in_=prs0[:])
                with nc.allow_low_precision(reason="f32r PE operand"):
                    nc.vector.tensor_mul(ctx[:, :QB], pctx0[:], rsbr[:, :QB])
                nc.vector.reciprocal_approx_fast(out=rsbr[:, QB:], in_=prs1[:])
                with nc.allow_low_precision(reason="f32r PE operand"):
                    nc.vector.tensor_mul(ctx[:, QB:], pctx1[:], rsbr[:, QB:])
                return ctx, carry

            def attn_batch(b, dmas, units, pre, front=0):
                uit = iter(units)
                nunits = len(units)
                emitted = [0]
                slot = [0]

                def pull():
                    want = (slot[0] + 1) * nunits // (8 * NKT2)
                    if front:
                        want = max(want, min(front, 3 * (slot[0] + 1)))
                    while emitted[0] < want:
                        next(uit)()
                        emitted[0] += 1
                    slot[0] += 1

                last_batch = b == B - 1
                if dmas:
                    dmas[0]()
                    dmas[1]()
                for qbp in range(2):
                    ctxs = []
                    for h in range(HPC):
                        s_idx = qbp * 2 + h
                        if s_idx + 2 < len(dmas):
                            dmas[s_idx + 2]()
                        if s_idx == 3:
                            # the cross-batch warm reads QT/KT of batch b+1:
                            # force the remaining projection units out first
                            # (they precede the warm in the PE FIFO).
                            for fn in uit:
                                fn()
                                emitted[0] += 1
                        if s_idx < 3:
                            nxt = (b, (h + 1) % HPC, qbp + (h == HPC - 1))
                        elif not last_batch:
                            nxt = (b + 1, 0, 0)
                        else:
                            nxt = None
                        ctx, pre = section(b, h, qbp, pull, pre, nxt, last_batch)
                        ctxs.append(ctx)
                    # output projection for this query pair-block
                    ytile = work.tile([128, 2 * QB], bf16, tag="ytile", name="ytile")
                    for half, hsl in ((0, slice(0, QB)), (1, slice(QB, 2 * QB))):
                        pz = psc.tile([128, QB], f32, tag="pC", name="pz")
                        for h in range(HPC):
                            nc.tensor.matmul(
                                pz[:],
                                wh_sb[:, h, :],
                                ctxs[h][:, hsl],
                                start=(h == 0),
                                stop=(h == HPC - 1),
                            )
                        with nc.allow_low_precision(reason="bf16 RS payload"):
                            nc.vector.tensor_scalar_add(
                                ytile[:, hsl], pz[:], biasE_sb[:, 0:1]
                            )
                        nc.gpsimd.dma_start(y_bounce[b][qbp][:, hsl], ytile[:, hsl])
                    nc.gpsimd.collective_compute(
                        "ReduceScatter",
                        mybir.AluOpType.add,
                        replica_groups=[list(range(NCORES))],
                        ins=[y_bounce[b][qbp][:].opt()],
                        outs=[y_shard[b][qbp][:].opt()],
                    )
                    nc.sync.dma_start(
                        out_y[:, b * S + qbp * (S // 2) : b * S + (qbp + 1) * (S // 2)],
                        y_shard[b][qbp][:],
                    )
                return pre

            # ---- prologue: only K(h0) + Q(h0, tb0/tb1) run serially; the
            # rest of batch 0's projection flows through the interleave
            # queue of its own first attention sections (V first, so the
            # inline PV matmuls always find V8 written; then K(h1)/Q(h1)
            # which the first section-boundary warm needs). ----
            dma_consts_early()
            _, _, uq0, uk0, uv0, dq0, dk0 = make_proj(0)
            dk0(0)
            dk0(1)
            dma_consts_late()
            dq0(0)
            dq0(1)
            dk0(2)
            dk0(3)
            dq0(2)
            dq0(3)
            for tb in range(NTB):
                for u in uk0[(tb, 0)]:
                    u()
            for tb in (0, 1):
                for u in uq0[(tb, 0)]:
                    u()

            # warm attn(0)'s very first scores subtile at the prologue end
            pre = pexpp.tile([128, NKT2, 2, 2 * QB], f8, tag="pexp", name="pexp")
            qk_sub(pre, 0, 0, 0, 0, 0)

            b0_queue = []
            for tb in range(NTB):
                b0_queue += uv0[(tb, 0)] + uv0[(tb, 1)]
            for tb in range(NTB):
                b0_queue += uk0[(tb, 1)]
            b0_queue += uq0[(0, 1)] + uq0[(1, 1)]
            b0_queue += uq0[(2, 0)] + uq0[(3, 0)] + uq0[(2, 1)] + uq0[(3, 1)]

            leftover, front = b0_queue, len(b0_queue)
            for b in range(B):
                if b + 1 < B:
                    dmas, units, _, _, _, _, _ = make_proj(b + 1)
                else:
                    dmas, units = [], []
                pre = attn_batch(b, dmas, leftover + units, pre, front=front)
                leftover, front = [], 0

    nc.compile()
    return nc


def kernel(**inputs):
    query = np.asarray(inputs["query"], np.float32)
    key = np.asarray(inputs["key"], np.float32)
    Wq, bq = np.asarray(inputs["Wq"], np.float32), np.asarray(inputs["bq"], np.float32)
    Wk, bk = np.asarray(inputs["Wk"], np.float32), np.asarray(inputs["bk"], np.float32)
    Wv = np.asarray(inputs["Wv"], np.float32)
    bv = np.asarray(inputs["bv"], np.float32)
    Wp, bp = np.asarray(inputs["Wp"], np.float32), np.asarray(inputs["bp"], np.float32)
    Wo, bo = np.asarray(inputs["Wo"], np.float32), np.asarray(inputs["bo"], np.float32)

    e4 = ml_dtypes.float8_e4m3fn if hasattr(ml_dtypes, "float8_e4m3fn") else ml_dtypes.float8_e4m3
    WS = 32.0

    qT_f8 = np.ascontiguousarray(query.transpose(0, 2, 1)).astype(e4)
    kT_f8 = np.ascontiguousarray(key.transpose(0, 2, 1)).astype(e4)

    if "nc" not in _cache:
        _cache["nc"] = build()
    nc = _cache["nc"]

    in_maps = []
    for i in range(NCORES):
        hs = slice(i * HPC, (i + 1) * HPC)
        Wo_h = Wo.reshape(H, D, D)  # rows of Wo per head
        wh = np.einsum(
            "hde,hef->hdf",
            Wp[hs].astype(np.float64),
            Wo_h[hs].astype(np.float64),
        ).astype(np.float32)
        bias = (
            np.einsum("hd,hdf->f", bv[hs].astype(np.float64), wh.astype(np.float64))
            + np.einsum(
                "hd,hdf->f", bp[hs].astype(np.float64), Wo_h[hs].astype(np.float64)
            )
            + bo.astype(np.float64) / NCORES
        ).astype(np.float32)
        in_maps.append(
            {
                "qT": qT_f8,
                "kT": kT_f8,
                "wq": np.ascontiguousarray(Wq[hs] * WS).astype(e4),
                "wk": np.ascontiguousarray(Wk[hs] * WS).astype(e4),
                "wv": np.ascontiguousarray(
                    (Wv[hs] * WS).reshape(HPC, NCH, 128, D).transpose(2, 1, 0, 3)
                ).astype(e4),
                "wh": wh / WS,
                "bqT": np.ascontiguousarray(bq[hs].T) * WS,
                "bkT": np.ascontiguousarray(bk[hs].T) * WS,
                "biasE": bias.reshape(D, 1),
                "ones8": np.ones((128, 2, 128), e4),
            }
        )

    res = run_bass_kernel_spmd(nc, in_maps, core_ids=list(range(NCORES)))
    _cache["last_result"] = res
    yT = np.concatenate(
        [np.asarray(res.results[i]["out_y"]).astype(np.float32) for i in range(NCORES)],
        axis=0,
    )
    return np.ascontiguousarray(yT.T).reshape(B, S, D)
